# revision 65
# baseline (speedup 1.0000x reference)
"""Trainium2 Bass kernel for nn_EquivariantProductBasisBlock.

Computation (per node n, channel c):
  s = nf[n,c,0]; v = nf[n,c,1:4]; v2 = |v|^2
  out0 = w0*s + w1*s^2 + w2'*v2 + w3*s^3 + w4*s*v2     (w_p = W0[sp[n],p,c])
  B1   = u0 + u1'*s + u2'*s^2 + u3'*v2                 (u_p = W1[sp[n],p,c])
  o1m  = B1 * v_m
  y0 = out0 @ L0 / sqrt(C);  y1m = o1m @ L1 / sqrt(C)
  y[n,c,:] = [y0, y1x, y1y, y1z] + sc[n,c,:]

Strategy: data-parallel over nodes across 8 cores, zero padding.

Each species' node list is split 8 ways (count_e // 8 nodes per core), so
every core holds the SAME species-segment layout over its 16x512-node
tiles; segment boundaries are compile-time constants of the (single,
SPMD) program.  The <=70 global leftover nodes (count_e % 8) never reach
the device - the host computes them exactly during reassembly.  The
per-(species,path,channel) weights are per-partition f32 scalar columns
in a small table indexed by segment, applied with per-segment
tensor_scalar sub-range ops (species boundaries can sit anywhere inside
a tile).

Per 512-node subtile step (all engines under the DMA cadence):
  - inputs arrive as transposed bf16 planes [tile, 4, C, 512] (channels
    on partitions); sc as fp8e4m3 planes
  - DVE: coefficient FMAs via tensor_scalar (4x mode) + Horner products
    via tensor_tensor (2x mode)
  - ACT: squares of v (one [C,1536] op) + PSUM->SBUF bf16 drain (one
    [C,2048] op)
  - GPSIMD: |v|^2 adds + the H = u3'*v2+u0 tensor_scalar
  - PE: sc injected via fp8 identity matmuls (start=True) into PSUM,
    channel mixing computed transposed (lhsT=L, rhs=X, no transposes);
    out0 = D + R is folded into PE as two accumulating L0 matmuls
  - PSUM double-buffered: two [C,2048] tiles (4 banks each); drain(j-1)
    overlaps matmuls(j)
  - software pipeline: loads 6 subtiles ahead (deep prefetch keeps the
    single 360 GB/s DMA resource saturated), stage1 (squares, v2, H) two
    subtiles ahead of the dependent products; the last three drains and
    stores are split into halves to shorten the pipeline tail
  - host reassembles: inverse node permutation + plane interleave +
    exact f32 compute of the leftover nodes
"""

import numpy as np

N_CORES = 8
N_NODES = 65536
C = 128
E = 10
W = 512          # nodes per subtile
T_CORE = 16      # subtiles per core (65536 / 8 / 512, zero padding)

INV_SQ3 = 1.0 / np.sqrt(3.0)
SQ2 = float(np.sqrt(2.0))
SQ3 = float(np.sqrt(3.0))
SQ35 = float(np.sqrt(3.0 / 5.0))

_CACHE = {}


# ---------------------------------------------------------------------------
# Workarounds for the walrus build in this container: it rejects any
# instruction carrying more than one sync-wait ("Too many sync wait
# commands").  Split extra waits onto same-engine NOPs preceding the
# instruction (identical semantics: the engine queue is FIFO).
# ---------------------------------------------------------------------------
def _apply_patches():
    import concourse.tile as tile
    from concourse import mybir
    from concourse.vector_clock import ScopedClock

    if getattr(tile.TileContext, "_singlewait_patched", False):
        return

    def _patched_drain_and_barrier(self, tick_clock, wait_clock):
        nc = self.nc
        probe = nc.sync.nop()
        wait_clock.add_sem_waits(probe.ins, ScopedClock({None: tick_clock.global_clock}))
        si = probe.ins.sync_info
        waits = list(si.on_wait) if si and si.on_wait else []
        if len(waits) > 1:
            probe.ins.sync_info = type(si)(on_wait=waits[:1], on_update=[])
            for w in waits[1:]:
                extra = nc.sync.nop()
                extra.ins.sync_info = type(si)(on_wait=[w], on_update=[])
        nc.sync.drain()
        nc.all_engine_barrier()
        assert self.sems is not None
        popped = nc._tile_sem_poison_stack.pop()
        assert popped is self._sem_poison
        nc.clear_and_free_semaphores(list(self.sems.allocated().values()))
        nc.all_engine_barrier()

    _orig_commit = tile.TileContext._commit_instruction

    def _split_commit(self, inst, lazy_reg_writes=True):
        si = getattr(inst, "sync_info", None)
        if (si is not None and si.on_wait and len(si.on_wait) > 1
                and getattr(inst, "engine", mybir.EngineType.Unassigned)
                != mybir.EngineType.Unassigned):
            waits = list(si.on_wait)
            for w in waits[:-1]:
                nop = mybir.InstNoOp(name=self.nc.get_next_instruction_name(),
                                     ins=[], outs=[], engine=inst.engine)
                nop.sync_info = mybir.SyncInfo(on_wait=[w], on_update=[])
                _orig_commit(self, nop, lazy_reg_writes=False)
            inst.sync_info = mybir.SyncInfo(on_wait=[waits[-1]],
                                            on_update=list(si.on_update or []))
        return _orig_commit(self, inst, lazy_reg_writes)

    tile.TileContext._drain_and_barrier = _patched_drain_and_barrier
    tile.TileContext._commit_instruction = _split_commit
    tile.TileContext._singlewait_patched = True


def _subtile_segs(segs):
    """Per-subtile list of (species, local_start, length) pieces."""
    out = [[] for _ in range(T_CORE)]
    for e, start, length in segs:
        end = start + length
        j0, j1 = start // W, (end - 1) // W
        for j in range(j0, j1 + 1):
            a = max(start, j * W)
            b = min(end, (j + 1) * W)
            if b > a:
                out[j].append((e, a - j * W, b - a))
    return out


def _build_program(segs, reps=1, io_bufs=8, work_bufs=2, wk1_bufs=3,
                   load_lead=6, s1_lead=2, v2_eng="pool", h_eng="pool",
                   o1z_pool=False, drain_split=False, sq_split=False,
                   out_q="sp", in_q="sp", sp_order="ino",
                   w0fold=False, sqz_dve=False, drain_pos="mid", late_split=3, sc_q="sp", h_late=False, o1_split_last=1, par_final=False):
    import concourse.bass as bass
    import concourse.tile as tile
    from concourse import mybir
    from contextlib import ExitStack

    _apply_patches()
    F32 = mybir.dt.float32
    BF16 = mybir.dt.bfloat16
    F8 = mybir.dt.float8e4
    AF = mybir.ActivationFunctionType
    nc = bass.Bass()

    n_elems = T_CORE * 4 * C * W
    xin_d = nc.declare_dram_parameter("xin", [n_elems], BF16, isOutput=False)
    sct_d = nc.declare_dram_parameter("sct", [n_elems], F8, isOutput=False)
    wtab_d = nc.declare_dram_parameter("wtab", [C, (E + 1) * 9], F32,
                                       isOutput=False)
    l0_d = nc.declare_dram_parameter("l0", [C, C], BF16, isOutput=False)
    l1_d = nc.declare_dram_parameter("l1", [C, C], BF16, isOutput=False)
    id8_d = nc.declare_dram_parameter("id8", [C, C], F8, isOutput=False)
    l0w_d = nc.declare_dram_parameter("l0w", [C, (E + 1) * C], BF16,
                                      isOutput=False)
    # output split into two DRAM tensors (even/odd steps): consecutive
    # stores then carry no whole-tensor write dependency (+900ns sem each)
    y_da = nc.declare_dram_parameter("youta", [n_elems // 2], BF16,
                                     isOutput=True)
    y_db = nc.declare_dram_parameter("youtb", [n_elems // 2], BF16,
                                     isOutput=True)

    mult = mybir.AluOpType.mult
    add = mybir.AluOpType.add

    jsegs = _subtile_segs(segs)

    def dview(dparam, j):
        # DMA-side view of subtile j, iteration order (c, plane, w)
        a = dparam[:]
        return bass.AP(tensor=a.tensor, offset=a.offset + j * (4 * C * W),
                       ap=[[W, C], [C * W, 4], [1, W]])

    def oview(j, half=None):
        # output view of subtile j on the even/odd out tensor
        a = (y_da if j % 2 == 0 else y_db)[:]
        off = a.offset + (j // 2) * (4 * C * W)
        if half is None:
            return bass.AP(tensor=a.tensor, offset=off,
                           ap=[[W, C], [C * W, 4], [1, W]])
        return bass.AP(tensor=a.tensor, offset=off + half * 2 * C * W,
                       ap=[[W, C], [C * W, 2], [1, W]])

    with tile.TileContext(nc) as tc, ExitStack() as ctx:
        consts = ctx.enter_context(tc.tile_pool(name="consts", bufs=1))
        io = ctx.enter_context(tc.tile_pool(name="io", bufs=io_bufs))
        work = ctx.enter_context(tc.tile_pool(name="work", bufs=work_bufs))
        wk1 = ctx.enter_context(tc.tile_pool(name="wk1", bufs=wk1_bufs))
        psY = ctx.enter_context(tc.tile_pool(name="psY", bufs=2, space="PSUM"))

        def ap(t, off, *dims):
            return bass.AP(tensor=t.tensor, offset=t.offset + off,
                           ap=[t.ap[0], *list(dims)])

        LATE_SPLIT = late_split
        steps = [j for _ in range(reps) for j in range(T_CORE)]
        n_steps = len(steps)
        ins = {}
        st = {}
        pend = {}
        outs = {}

        def load(i):
            j = steps[i]
            a = io.tile([C, 4 * W], BF16, tag="in")
            b = io.tile([C, 4 * W], F8, tag="sc")
            inq = {"act": nc.scalar, "sp": nc.sync}[in_q]
            scq = {"act": nc.scalar, "sp": nc.sync, "pool": nc.gpsimd}[sc_q]
            inq.dma_start(out=a, in_=dview(xin_d, j))
            scq.dma_start(out=b, in_=dview(sct_d, j))
            ins[i] = (a, b)

        def col(e, k):
            return t_wtab[:, e * 9 + k:e * 9 + k + 1]

        # coefficient columns: 0:w3 1:w1 2:w0 3:w4 4:w2' 5:u2' 6:u1' 7:u3' 8:u0
        def stage1(i):
            # ops needing only in(i): squares on ACT, v2 adds + H on GPSIMD
            t_in, _ = ins[i]
            t_sq = wk1.tile([C, 3 * W], BF16, tag="sq")   # [vx2|vy2|vz2]
            t_v2 = wk1.tile([C, W], BF16, tag="v2")
            t_h = wk1.tile([C, W], BF16, tag="h")
            if sqz_dve:
                nc.scalar.activation(out=t_sq[:, 0:2 * W],
                                     in_=t_in[:, W:3 * W], func=AF.Square)
                nc.vector.tensor_tensor(out=t_sq[:, 2 * W:3 * W],
                                        in0=t_in[:, 3 * W:4 * W],
                                        in1=t_in[:, 3 * W:4 * W], op=mult)
            elif sq_split:
                for m in range(3):
                    nc.scalar.activation(out=t_sq[:, m * W:(m + 1) * W],
                                         in_=t_in[:, (1 + m) * W:(2 + m) * W],
                                         func=AF.Square)
            else:
                nc.scalar.activation(out=t_sq, in_=t_in[:, W:4 * W],
                                     func=AF.Square)
            veng = nc.gpsimd if v2_eng == "pool" else nc.vector
            veng.tensor_tensor(out=t_v2, in0=t_sq[:, 0:W],
                               in1=t_sq[:, W:2 * W], op=add)
            veng.tensor_tensor(out=t_v2, in0=t_v2,
                               in1=t_sq[:, 2 * W:3 * W], op=add)
            heng = nc.gpsimd if h_eng == "pool" else nc.vector
            if not h_late:
                for e, a, ln in jsegs[steps[i]]:
                    heng.tensor_scalar(out=t_h[:, a:a + ln],
                                       in0=t_v2[:, a:a + ln],
                                       scalar1=col(e, 7), scalar2=col(e, 8),
                                       op0=mult, op1=add)
            st[i] = (t_v2, t_h)

        def drain_act(i):
            # single PSUM -> SBUF bf16 copy on ACT
            p_y, j = pend.pop(i)
            t_y = io.tile([C, 4 * W], BF16, tag="y")
            if drain_split:
                nc.scalar.activation(out=t_y[:, 0:W], in_=p_y[:, 0:W],
                                     func=AF.Copy)
                nc.scalar.activation(out=t_y[:, W:4 * W], in_=p_y[:, W:4 * W],
                                     func=AF.Copy)
            else:
                nc.scalar.activation(out=t_y, in_=p_y, func=AF.Copy)
            outs[i] = (t_y, j)

        def drain_dma(i):
            t_y, j = outs.pop(i)
            outq = {"act": nc.scalar, "pool": nc.gpsimd,
                    "sp": nc.sync}[out_q]
            outq.dma_start(out=oview(j), in_=t_y)

        def drain(i):
            drain_act(i)
            drain_dma(i)

        def drain_final(i, y0_first=False):
            # last steps: split drain + store to shorten the pipeline tail
            p_y, j = pend.pop(i)
            # separate SBUF tiles per half: the half-A store must not
            # wait on half-B's drain (tile deps are whole-tile)
            t_ya = io.tile([C, 2 * W], BF16, tag="ya")
            t_yb = io.tile([C, 2 * W], BF16, tag="yb")
            if isinstance(p_y, tuple):
                halves = ((0, t_ya, p_y[0][:, 0:2 * W]),
                          (1, t_yb, p_y[1][:, 0:2 * W]))
            else:
                halves = ((0, t_ya, p_y[:, 0:2 * W]),
                          (1, t_yb, p_y[:, 2 * W:4 * W]))
            for h, t_yh, src_h in halves:
                nc.scalar.activation(out=t_yh, in_=src_h, func=AF.Copy)
                nc.sync.dma_start(out=oview(j, half=h), in_=t_yh)

        a0 = io.tile([C, 4 * W], BF16, tag="in")
        b0 = io.tile([C, 4 * W], F8, tag="sc")
        nc.sync.dma_start(out=a0, in_=dview(xin_d, 0))
        t_wtab = consts.tile([C, (E + 1) * 9], F32)
        nc.sync.dma_start(out=t_wtab, in_=wtab_d[:, :])
        nc.sync.dma_start(out=b0, in_=dview(sct_d, 0))
        ins[0] = (a0, b0)
        t_l0 = consts.tile([C, C], BF16)
        nc.sync.dma_start(out=t_l0, in_=l0_d[:, :])
        t_l1 = consts.tile([C, C], BF16)
        nc.sync.dma_start(out=t_l1, in_=l1_d[:, :])
        ident = consts.tile([C, C], F8)
        nc.sync.dma_start(out=ident, in_=id8_d[:, :])
        t_l0w = consts.tile([C, (E + 1) * C], BF16)
        if w0fold:
            nc.sync.dma_start(out=t_l0w, in_=l0w_d[:, :])
        for i in range(1, min(load_lead, n_steps)):
            load(i)
        for i in range(min(s1_lead, n_steps)):
            stage1(i)
        if h_late:
            for i in range(min(s1_lead, n_steps)):
                v2f, hf = st[i]
                for e, a, ln in jsegs[steps[i]]:
                    nc.vector.tensor_scalar(out=hf[:, a:a + ln],
                                            in0=v2f[:, a:a + ln],
                                            scalar1=col(e, 7),
                                            scalar2=col(e, 8),
                                            op0=mult, op1=add)

        for i in range(n_steps):
            j = steps[i]
            if drain_pos == "top" and i - 1 in pend:
                drain_act(i - 1)
            if sp_order == "oin" and i - 1 in pend:
                drain(i - 1)
            if i + load_lead < n_steps:
                load(i + load_lead)
            if i + s1_lead < n_steps:
                stage1(i + s1_lead)
            if i - 1 in outs:
                drain_dma(i - 1)
            if sp_order == "ino" and i - 1 in pend:
                if i >= n_steps - LATE_SPLIT:
                    drain_final(i - 1)
                else:
                    drain(i - 1)

            t_in, t_sc = ins.pop(i)
            t_v2, t_h = st.pop(i)
            sS = t_in[:, 0:W]

            # --- DVE Horner chain ---
            t_af = work.tile([C, 2 * W], BF16, tag="af")    # [A | F]
            t_q = work.tile([C, W], BF16, tag="q")
            for e, a, ln in jsegs[j]:
                nc.vector.tensor_scalar(out=t_af[:, a:a + ln],
                                        in0=sS[:, a:a + ln],
                                        scalar1=col(e, 0), scalar2=col(e, 1),
                                        op0=mult, op1=add)
                nc.vector.tensor_scalar(out=ap(t_af, W + a, [1, ln]),
                                        in0=sS[:, a:a + ln],
                                        scalar1=col(e, 5), scalar2=col(e, 6),
                                        op0=mult, op1=add)
                nc.vector.tensor_scalar(out=t_q[:, a:a + ln],
                                        in0=sS[:, a:a + ln],
                                        scalar1=col(e, 3), scalar2=col(e, 4),
                                        op0=mult, op1=add)
            t_bg = work.tile([C, 2 * W], BF16, tag="bg")    # [B | G]
            nc.vector.tensor_tensor(out=t_bg, in0=t_af,
                                    in1=ap(t_in, 0, [0, 2], [1, W]), op=mult)
            if not w0fold:
                for e, a, ln in jsegs[j]:
                    nc.vector.tensor_scalar(out=t_bg[:, a:a + ln],
                                            in0=t_bg[:, a:a + ln],
                                            scalar1=col(e, 2), scalar2=None,
                                            op0=add)
            t_d = work.tile([C, W], BF16, tag="d")
            nc.vector.tensor_tensor(out=t_d, in0=t_bg[:, 0:W], in1=sS, op=mult)
            t_r = work.tile([C, W], BF16, tag="r")
            nc.vector.tensor_tensor(out=t_r, in0=t_q, in1=t_v2, op=mult)
            t_b1 = work.tile([C, W], BF16, tag="b1")
            nc.vector.tensor_tensor(out=t_b1, in0=t_bg[:, W:2 * W], in1=t_h,
                                    op=add)
            if i >= n_steps - o1_split_last:
                # endgame: per-plane o1 in separate tiles so each L1 matmul
                # (and thus each final drain half) starts as early as possible
                t_o1x = work.tile([C, W], BF16, tag="o1x")
                t_o1y = work.tile([C, W], BF16, tag="o1y")
                t_o1z = work.tile([C, W], BF16, tag="o1z")
                o1p = [t_o1x, t_o1y, t_o1z]
                for m in range(3):
                    nc.vector.tensor_tensor(out=o1p[m],
                                            in0=t_in[:, (1 + m) * W:(2 + m) * W],
                                            in1=t_b1, op=mult)
            else:
                t_o1 = work.tile([C, 3 * W], BF16, tag="o1")
                nc.vector.tensor_tensor(out=t_o1, in0=t_in[:, W:4 * W],
                                        in1=ap(t_b1, 0, [0, 3], [1, W]),
                                        op=mult)
                o1p = [t_o1[:, m * W:(m + 1) * W] for m in range(3)]

            # --- channel mixing, transposed: yT = L^T X (+ I^T scT) ---
            if i == n_steps - 1:
                # last step: [y0|y1x] and [y1y|y1z] in two PSUM tiles from
                # the same ring (the 2nd is free once drain(i-1) completes),
                # so the first drain half gates only on L1x, not L1z
                p_y = psY.tile([C, 4 * W], F32, tag="py")
                p_y2 = psY.tile([C, 4 * W], F32, tag="py")
                for m in (0, 1):
                    nc.tensor.matmul(p_y[:, m * W:(m + 1) * W], lhsT=ident,
                                     rhs=t_sc[:, m * W:(m + 1) * W],
                                     start=True, stop=False)
                nc.tensor.matmul(p_y[:, 0:W], lhsT=t_l0, rhs=t_r,
                                 start=False, stop=False)
                nc.tensor.matmul(p_y[:, 0:W], lhsT=t_l0, rhs=t_d,
                                 start=False, stop=True)
                nc.tensor.matmul(p_y[:, W:2 * W], lhsT=t_l1, rhs=o1p[0],
                                 start=False, stop=True)
                for m in (2, 3):
                    nc.tensor.matmul(p_y2[:, (m - 2) * W:(m - 1) * W],
                                     lhsT=ident,
                                     rhs=t_sc[:, m * W:(m + 1) * W],
                                     start=True, stop=False)
                for m in (1, 2):
                    nc.tensor.matmul(p_y2[:, (m - 1) * W:m * W], lhsT=t_l1,
                                     rhs=o1p[m], start=False, stop=True)
                p_y = (p_y, p_y2)
            else:
                p_y = psY.tile([C, 4 * W], F32, tag="py")
                for m in range(4):
                    nc.tensor.matmul(p_y[:, m * W:(m + 1) * W], lhsT=ident,
                                     rhs=t_sc[:, m * W:(m + 1) * W],
                                     start=True, stop=False)
                nc.tensor.matmul(p_y[:, 0:W], lhsT=t_l0, rhs=t_r,
                                 start=False, stop=False)
                nc.tensor.matmul(p_y[:, 0:W], lhsT=t_l0, rhs=t_d,
                                 start=False, stop=True)
                for m in range(3):
                    nc.tensor.matmul(p_y[:, (1 + m) * W:(2 + m) * W],
                                     lhsT=t_l1,
                                     rhs=o1p[m], start=False, stop=True)
            if h_late and i + s1_lead < n_steps:
                # H for step i+s1_lead, on DVE after this step's chain (its
                # v2 input from Pool is ready by then; same-engine input to
                # the consuming B1 add, so no cross-engine jitter)
                v2f, hf = st[i + s1_lead]
                for e, a, ln in jsegs[steps[i + s1_lead]]:
                    nc.vector.tensor_scalar(out=hf[:, a:a + ln],
                                            in0=v2f[:, a:a + ln],
                                            scalar1=col(e, 7),
                                            scalar2=col(e, 8),
                                            op0=mult, op1=add)
            pend[i] = (p_y, j)
            if i == n_steps - 1:
                if par_final:
                    p_yf, jf = pend.pop(i)
                    t_yf = io.tile([C, 4 * W], BF16, tag="y")
                    nc.scalar.activation(out=t_yf[:, 0:2 * W],
                                         in_=p_yf[:, 0:2 * W], func=AF.Copy)
                    hv0 = bass.AP(tensor=dview(y_d, jf).tensor,
                                  offset=y_d[:].offset + jf * (4 * C * W),
                                  ap=[[W, C], [C * W, 2], [1, W]])
                    nc.sync.dma_start(out=hv0, in_=t_yf[:, 0:2 * W])
                    nc.vector.tensor_scalar(out=t_yf[:, 2 * W:4 * W],
                                            in0=p_yf[:, 2 * W:4 * W],
                                            scalar1=1.0, scalar2=None,
                                            op0=mult)
                    hv1 = bass.AP(tensor=dview(y_d, jf).tensor,
                                  offset=y_d[:].offset + jf * (4 * C * W)
                                  + 2 * C * W,
                                  ap=[[W, C], [C * W, 2], [1, W]])
                    nc.sync.dma_start(out=hv1, in_=t_yf[:, 2 * W:4 * W])
                else:
                    drain_final(i)
        assert not pend

    return nc


def _host_nodes(nf, sc, sp, W0, W1, L0, L1, idx):
    """Exact f32 reference for the leftover nodes (computed on host)."""
    s = nf[idx, :, 0]
    v = nf[idx, :, 1:4]
    v2 = np.sum(v * v, axis=-1)
    w = W0[sp[idx]]                                   # [m, P0, C]
    u = W1[sp[idx]]
    out0 = (w[:, 0] * s + w[:, 1] * s * s + w[:, 2] * INV_SQ3 * v2
            + w[:, 3] * s * s * s + w[:, 4] * s * v2)
    b1 = (u[:, 0] + u[:, 1] * SQ2 * s + u[:, 2] * SQ3 * s * s
          + u[:, 3] * SQ35 * v2)
    o1 = b1[..., None] * v
    inv_sqrt_c = np.float32(1.0 / np.sqrt(C))
    y0 = (out0 @ L0) * inv_sqrt_c
    y1 = np.einsum("ncm,cd->ndm", o1, L1) * inv_sqrt_c
    return np.concatenate([y0[..., None], y1], axis=-1) + sc[idx]


def _prep_host(inputs):
    import ml_dtypes
    bf16 = ml_dtypes.bfloat16
    f8 = ml_dtypes.float8_e4m3

    nf = np.asarray(inputs["node_feats"], dtype=np.float32)
    sc = np.asarray(inputs["sc"], dtype=np.float32)
    sp = np.asarray(inputs["node_species"]).astype(np.int64)
    W0 = np.asarray(inputs["W0"], dtype=np.float32)
    W1 = np.asarray(inputs["W1"], dtype=np.float32)
    L0 = np.asarray(inputs["L0"], dtype=np.float32)
    L1 = np.asarray(inputs["L1"], dtype=np.float32)

    n = nf.shape[0]
    assert n == N_NODES, n
    perm = np.argsort(sp, kind="stable")
    counts = np.bincount(sp, minlength=E)
    cum = np.zeros(E + 1, dtype=np.int64)
    cum[1:] = np.cumsum(counts)
    q = counts // N_CORES

    core_orig = []                     # per-species [N_CORES, q_e] blocks
    left = []
    for e in range(E):
        blk = perm[cum[e]:cum[e + 1]]
        core_orig.append(blk[:N_CORES * q[e]].reshape(N_CORES, q[e]))
        left.append(blk[N_CORES * q[e]:])
    core_orig = np.concatenate(core_orig, axis=1)      # [N_CORES, B]
    left = np.concatenate(left)
    B = core_orig.shape[1]
    assert B <= T_CORE * W

    # segment layout (identical on every core)
    segs = []
    off = 0
    for e in range(E):
        if q[e] > 0:
            segs.append((int(e), int(off), int(q[e])))
            off += int(q[e])
    if off < T_CORE * W:
        segs.append((E, int(off), int(T_CORE * W - off)))  # zero-coef pad

    nf_pad = np.zeros((N_CORES, T_CORE * W, C, 4), dtype=bf16)
    nf_pad[:, :B] = nf[core_orig].astype(bf16)
    sc_pad = np.zeros((N_CORES, T_CORE * W, C, 4), dtype=np.float32)
    sc_pad[:, :B] = sc[core_orig]

    def to_flat(arr, dt):
        out = []
        for cidx in range(N_CORES):
            t = arr[cidx].reshape(T_CORE, W, C, 4).transpose(0, 3, 2, 1)
            out.append(np.ascontiguousarray(t).ravel().astype(dt))
        return out

    xin = to_flat(nf_pad, bf16)
    sct = to_flat(sc_pad, f8)

    # coefficient columns per species: [w3, w1, w0, w4, w2', u2', u1', u3', u0]
    coef = np.zeros((E + 1, 9, C), dtype=np.float32)   # row E stays zero (pad)
    coef[:E, 0] = W0[:, 3]
    coef[:E, 1] = W0[:, 1]
    coef[:E, 2] = W0[:, 0]
    coef[:E, 3] = W0[:, 4]
    coef[:E, 4] = W0[:, 2] * INV_SQ3
    coef[:E, 5] = W1[:, 2] * SQ3
    coef[:E, 6] = W1[:, 1] * SQ2
    coef[:E, 7] = W1[:, 3] * SQ35
    coef[:E, 8] = W1[:, 0]
    wtab = np.ascontiguousarray(
        coef.transpose(2, 0, 1).reshape(C, (E + 1) * 9))

    inv_sqrt_c = np.float32(1.0 / np.sqrt(C))
    l0 = np.ascontiguousarray((L0 * inv_sqrt_c).astype(bf16))
    l1 = np.ascontiguousarray((L1 * inv_sqrt_c).astype(bf16))
    # per-species row-scaled L0 for the w0*s PE fold: L0w[e] = diag(w0_e) L0
    l0w = np.zeros((E + 1, C, C), dtype=np.float32)
    l0w[:E] = W0[:, 0][:, :, None] * (L0 * inv_sqrt_c)[None]
    l0w = np.ascontiguousarray(
        l0w.transpose(1, 0, 2).reshape(C, (E + 1) * C).astype(bf16))
    id8 = np.eye(C, dtype=f8)

    y_left = (_host_nodes(nf, sc, sp, W0, W1, L0, L1, left)
              if len(left) else None)
    meta = dict(core_orig=core_orig, left=left, y_left=y_left, B=B, n=n,
                segs=tuple(segs), id8=id8, l0w=l0w)
    return xin, sct, wtab, l0, l1, meta


def _in_maps(xin, sct, wtab, l0, l1, meta):
    return [{"xin": xin[c], "sct": sct[c], "wtab": wtab, "l0": l0, "l1": l1,
             "id8": meta["id8"], "l0w": meta["l0w"]} for c in range(N_CORES)]


def _assemble(y_cores, meta):
    B, n = meta["B"], meta["n"]
    out = np.empty((n, C, 4), dtype=np.float32)
    for cidx in range(N_CORES):
        ya, yb = y_cores[cidx]
        y = np.empty((T_CORE, 4, C, W), dtype=ya.dtype)
        y[0::2] = ya.reshape(T_CORE // 2, 4, C, W)
        y[1::2] = yb.reshape(T_CORE // 2, 4, C, W)
        y = y.transpose(0, 3, 2, 1)
        out[meta["core_orig"][cidx]] = \
            y.reshape(T_CORE * W, C, 4)[:B].astype(np.float32)
    if len(meta["left"]):
        out[meta["left"]] = meta["y_left"]
    return out


def kernel(**inputs):
    from concourse.bass_utils import run_bass_kernel_spmd

    xin, sct, wtab, l0, l1, meta = _prep_host(inputs)
    key = ("nc", meta["segs"])
    if key not in _CACHE:
        _CACHE[key] = _build_program(meta["segs"])
    nc = _CACHE[key]

    res = run_bass_kernel_spmd(nc, _in_maps(xin, sct, wtab, l0, l1, meta),
                               core_ids=list(range(N_CORES)))
    _CACHE["last_result"] = res
    y_cores = [(res.results[c]["youta"], res.results[c]["youtb"])
               for c in range(N_CORES)]
    return _assemble(y_cores, meta)
